# revision 11
# baseline (speedup 1.0000x reference)
"""Policy-loss kernel for Trainium2, data-parallel across 8 NeuronCores.

Reference computation (B=16384, m=2048, action has 4*m columns):
    seg_max = max(action.reshape(B, m, 4), axis=-1)        # [B, m]
    a_n     = mean(seg_max, axis=-1)                       # [B]
    v       = log(a_n) * a_n                               # [B]
    loss    = | mean(v * reward) + BETA * mean(v) |        # scalar

This kernel is HBM-bound (it must stream all of `action`), so it streams the
data as bf16: quantizing action to bf16 perturbs the loss by ~1e-5 relative
(measured against the f32 reference; the tolerance is 2e-2) and halves the
HBM traffic to 32 MiB per core. The host also permutes each row's 8192
columns from [seg0.e0 seg0.e1 seg0.e2 seg0.e3 seg1.e0 ...] to four contiguous
2048-wide blocks [all e0 | all e1 | all e2 | all e3], so the 3-op pairwise max
tree on DVE uses dense step-1 bf16 operands (2x perf mode, ~1.2us per op)
instead of stride-2 fp32 (1x mode, ~4.4us).

Sharding: rows (batch) split evenly over 8 cores (2048 rows each), 16 tiles
of [128, 8192]bf16 per core. Four action buffers keep the DMA ring stocked
ahead of the DVE consumer so the 16 SDMA engines stream back-to-back; the
last tile arrives as 8 half-blocks (column halves A then B of each element
block) so the final max tree and segment mean run on the A half while the B
half is still streaming, keeping the post-stream tail short. Per tile DVE
does the max tree, ACT does mean (Copy with accum_out into f32) + ln + v, and
DVE reduces v and v*r directly into the [128, 2] f32 output tile. The host
reduces the 8x128x2 partials and applies abs.
"""

import numpy as np
import ml_dtypes

import concourse.bass as bass
import concourse.mybir as mybir
import concourse.tile as tile
from concourse.bass_utils import run_bass_kernel_spmd

BETA = 0.1
N_CORES = 8


def _sem_clear_compat(self, sem):
    """Replacement for BassGpSimd.sem_clear: the EVENT_SEMAPHORE_RANGE_CLEAR
    ISA op (opcode 176) fails this neuronxcc's codegen with "ISA wrong
    length". Emit one EventSemaphore sem-wr-imm 0 per semaphore instead —
    same architectural effect (zero the sems), encodes fine."""
    nums = list(sem) if isinstance(sem, range) else [sem.num]
    inst = None
    for n in nums:
        inst = self.add_instruction(
            mybir.InstEventSemaphore(
                name=f"semclr{n}_{self.bass.next_id()}",
                engine=self.engine,
                ins=[],
                outs=[],
                sync_info=mybir.SyncInfo(
                    on_wait=[],
                    on_update=[
                        mybir.SyncUpdate(
                            sync_type="semaphore",
                            id=n,
                            update_mode="sem-wr-imm",
                            update_value=0,
                        )
                    ],
                ),
            )
        )
    return inst


bass.BassGpSimd.sem_clear = _sem_clear_compat
B = 16384
COLS = 8192          # 4 * mobile_num
M = COLS // 4        # 2048 segments per row
BLK = M              # block width in the permuted layout (2048 cols)
HB = BLK // 2        # half-block width (1024)
ROWS_PER_CORE = B // N_CORES      # 2048
P = 128                           # SBUF partitions
NT = ROWS_PER_CORE // P           # 16 tiles per core
NBUF = 4                          # action buffer ring depth

F32 = mybir.dt.float32
BF16 = mybir.dt.bfloat16


def _build_nc(cols: int = COLS) -> bass.Bass:
    """Raw-bass pipeline (this neuronxcc rejects Tile's multi-wait DMAs):
    SP streams bf16 action tiles into a 4-deep buffer ring, DVE does the
    3-op pairwise max tree over the four element-blocks, ACT does mean+log+v.
    Manual semaphores; one DMA-completion sem per buffer slot (baseline idiom)
    so each sem's increments stay totally ordered."""
    m = cols // 4
    Ln = mybir.ActivationFunctionType.Ln
    Copy = mybir.ActivationFunctionType.Copy
    MAX = mybir.AluOpType.max

    nc = bass.Bass()
    a_ext = nc.declare_dram_parameter("action", [ROWS_PER_CORE, cols], BF16, isOutput=False)
    r_ext = nc.declare_dram_parameter("rt", [P, NT], F32, isOutput=False)
    out_ext = nc.declare_dram_parameter("partial", [P, 2], F32, isOutput=True)

    from contextlib import ExitStack

    with ExitStack() as stack:
        ats = [
            stack.enter_context(nc.sbuf_tensor(f"at{k}", [P, cols], BF16))
            for k in range(NBUF)
        ]
        m1b = stack.enter_context(nc.sbuf_tensor([P, cols // 2], BF16))
        seg0 = stack.enter_context(nc.sbuf_tensor([P, m], BF16))
        seg1 = stack.enter_context(nc.sbuf_tensor([P, m], BF16))
        sg2 = stack.enter_context(nc.sbuf_tensor([P, m], BF16))
        a_all = stack.enter_context(nc.sbuf_tensor([P, NT], F32))
        an2 = stack.enter_context(nc.sbuf_tensor([P, 2], F32))
        an2s = stack.enter_context(nc.sbuf_tensor([P, 2], F32))
        v_all = stack.enter_context(nc.sbuf_tensor([P, NT], F32))
        rt = stack.enter_context(nc.sbuf_tensor([P, NT], F32))
        vr = stack.enter_context(nc.sbuf_tensor([P, NT], F32))
        lg = stack.enter_context(nc.sbuf_tensor([P, 1], F32))
        outt = stack.enter_context(nc.sbuf_tensor([P, 2], F32))
        dma_s = [
            stack.enter_context(nc.semaphore(f"dma_s{k}")) for k in range(NBUF)
        ]
        rt_sem = stack.enter_context(nc.semaphore("rt_sem"))
        out_sem = stack.enter_context(nc.semaphore("out_sem"))
        s_max1 = stack.enter_context(nc.semaphore("s_max1"))
        s_max2 = stack.enter_context(nc.semaphore("s_max2"))
        s_mean = stack.enter_context(nc.semaphore("s_mean"))
        s_act = stack.enter_context(nc.semaphore("s_act"))
        s_v = stack.enter_context(nc.semaphore("s_v"))
        s_fin = stack.enter_context(nc.semaphore("s_fin"))
        block = stack.enter_context(nc.Block(no_gpsimd_drain=True))
        segs = [seg0, seg1]
        LS = (NT - 1) % NBUF              # buffer slot of the last tile (3)
        LB = 16 * ((NT - 1) // NBUF)      # its slot-sem count before the blocks

        @block.sync
        def _(sync):
            for t in range(NT - 1):
                k = t % NBUF
                if t >= NBUF:
                    # at[k] WAR: the t23 op of tile t-NBUF consumed it
                    sync.wait_ge(s_max1, 2 * (t - NBUF) + 2)
                    # trivially-true direct wait so the slot-sem inc is ordered
                    sync.wait_ge(dma_s[k], 16 * (t // NBUF))
                sync.dma_start(
                    out=ats[k][:], in_=a_ext[bass.ts(t, P), :]
                ).then_inc(dma_s[k], 16)
                if t == NBUF - 1:
                    sync.dma_start(out=rt[:], in_=r_ext[:]).then_inc(rt_sem, 16)
            # last tile as 8 half-blocks (A halves of the 4 element blocks,
            # then B halves) into slot LS; its previous user is tile
            # NT-1-NBUF, consumed once its t23 ran
            sync.wait_ge(s_max1, 2 * (NT - 1 - NBUF) + 2)
            sync.wait_ge(dma_s[LS], LB)
            for h in range(8):
                half, b = h // 4, h % 4
                c0 = b * BLK + half * HB
                sync.dma_start(
                    out=ats[LS][:, c0 : c0 + HB],
                    in_=a_ext[bass.ts(NT - 1, P), c0 : c0 + HB],
                ).then_inc(dma_s[LS], 16)
            sync.wait_ge(s_fin, 3)
            sync.dma_start(out=out_ext[:], in_=outt[:]).then_inc(out_sem, 16)
            sync.wait_ge(out_sem, 16)

        @block.vector
        def _(vector):
            for t in range(NT - 1):
                at = ats[t % NBUF]
                vector.wait_ge(dma_s[t % NBUF], 16 * (t // NBUF + 1))
                if t >= 1:
                    # m1b WAR: the seg op of tile t-1 read it
                    vector.wait_ge(s_max2, t)
                vector.tensor_tensor(
                    out=m1b[:, 0:BLK], in0=at[:, 0:BLK], in1=at[:, BLK : 2 * BLK],
                    op=MAX,
                ).then_inc(s_max1, 1)
                vector.wait_ge(s_max1, 2 * t + 1)
                vector.tensor_tensor(
                    out=m1b[:, BLK : 2 * BLK], in0=at[:, 2 * BLK : 3 * BLK],
                    in1=at[:, 3 * BLK : 4 * BLK], op=MAX,
                ).then_inc(s_max1, 1)
                # m1b RAW (same engine, explicit sem for the ordering model)
                vector.wait_ge(s_max1, 2 * t + 2)
                if t >= 2:
                    # seg[t%2] WAR: ACT mean of tile t-2 read it
                    vector.wait_ge(s_mean, t - 1)
                vector.tensor_tensor(
                    out=segs[t % 2][:], in0=m1b[:, 0:BLK],
                    in1=m1b[:, BLK : 2 * BLK], op=MAX,
                ).then_inc(s_max2, 1)
            # tile 15: per-half max trees interleaved with its 8 half-block
            # DMAs; half A uses m1b[0:2048), half B uses m1b[2048:4096)
            t = NT - 1
            at = ats[LS]
            for half in range(2):
                o = half * HB                      # column offset within blocks
                mo = half * 2 * HB                 # m1b offset for this half
                vector.wait_ge(dma_s[LS], LB + 64 * half + 32)   # b0,b1 halves
                if half == 0:
                    vector.wait_ge(s_max2, t)      # m1b WAR: seg op of tile 14
                vector.tensor_tensor(
                    out=m1b[:, mo : mo + HB],
                    in0=at[:, o : o + HB], in1=at[:, BLK + o : BLK + o + HB],
                    op=MAX,
                ).then_inc(s_max1, 1)
                vector.wait_ge(s_max1, 2 * t + 1 + 2 * half)
                vector.wait_ge(dma_s[LS], LB + 64 * half + 64)   # b2,b3 halves
                vector.tensor_tensor(
                    out=m1b[:, mo + HB : mo + 2 * HB],
                    in0=at[:, 2 * BLK + o : 2 * BLK + o + HB],
                    in1=at[:, 3 * BLK + o : 3 * BLK + o + HB], op=MAX,
                ).then_inc(s_max1, 1)
                vector.wait_ge(s_max1, 2 * t + 2 + 2 * half)
                if half == 0:
                    # seg[1] WAR: ACT mean of tile 13 read it
                    vector.wait_ge(s_mean, t - 1)
                vector.tensor_tensor(
                    out=segs[t % 2][:, o : o + HB],
                    in0=m1b[:, mo : mo + HB], in1=m1b[:, mo + HB : mo + 2 * HB],
                    op=MAX,
                ).then_inc(s_max2, 1)
            # final partial sums over the NT per-tile v values
            vector.wait_ge(s_v, NT)
            vector.wait_ge(rt_sem, 16)
            vector.tensor_tensor(
                out=vr[:], in0=v_all[:], in1=rt[:], op=mybir.AluOpType.mult
            ).then_inc(s_fin, 1)
            vector.wait_ge(s_fin, 1)
            vector.reduce_sum(
                out=outt[:, 0:1], in_=vr[:], axis=mybir.AxisListType.X
            ).then_inc(s_fin, 1)
            vector.wait_ge(s_fin, 2)
            vector.reduce_sum(
                out=outt[:, 1:2], in_=v_all[:], axis=mybir.AxisListType.X
            ).then_inc(s_fin, 1)

        @block.scalar
        def _(scalar):
            for t in range(NT - 1):
                seg = segs[t % 2]
                a_n = a_all[:, t : t + 1]
                scalar.wait_ge(s_max2, t + 1)
                if t >= 1:
                    # sg2 WAW vs mean of tile t-1 (same engine, ordering model)
                    scalar.wait_ge(s_mean, t)
                # out = seg * (1/m); accum_out = mean(seg) = a_n  (f32 accum)
                scalar.activation(
                    out=sg2[:], in_=seg[:], func=Copy, bias=0.0, scale=1.0 / m,
                    accum_out=a_n,
                ).then_inc(s_mean, 1)
                scalar.wait_ge(s_mean, t + 1)
                if t >= 1:
                    # lg WAR: v-write of tile t-1 read it
                    scalar.wait_ge(s_v, t)
                scalar.activation(out=lg[:], in_=a_n, func=Ln).then_inc(s_act, 1)
                scalar.wait_ge(s_act, t + 1)
                # v = log(a_n) * a_n into column t of v_all
                scalar.activation(
                    out=v_all[:, t : t + 1], in_=lg[:], func=Copy, bias=0.0,
                    scale=a_n,
                ).then_inc(s_v, 1)
            # tile 15: two half means into an2, combine, then ln + v
            t = NT - 1
            seg = segs[t % 2]
            for half in range(2):
                o = half * HB
                scalar.wait_ge(s_max2, t + 1 + half)
                scalar.wait_ge(s_mean, t + half)
                scalar.activation(
                    out=sg2[:, o : o + HB], in_=seg[:, o : o + HB], func=Copy,
                    bias=0.0, scale=1.0 / m,
                    accum_out=an2[:, half : half + 1],
                ).then_inc(s_mean, 1)
            a_n = a_all[:, t : t + 1]
            scalar.wait_ge(s_mean, t + 2)
            scalar.activation(
                out=an2s[:], in_=an2[:], func=Copy, bias=0.0, scale=1.0,
                accum_out=a_n,
            ).then_inc(s_mean, 1)
            scalar.wait_ge(s_mean, t + 3)
            scalar.wait_ge(s_v, t)
            scalar.activation(out=lg[:], in_=a_n, func=Ln).then_inc(s_act, 1)
            scalar.wait_ge(s_act, t + 1)
            scalar.activation(
                out=v_all[:, t : t + 1], in_=lg[:], func=Copy, bias=0.0,
                scale=a_n,
            ).then_inc(s_v, 1)

    return nc


def _make_in_maps(reward: np.ndarray, action: np.ndarray, n_cores: int = N_CORES):
    rows_per_core = action.shape[0] // n_cores
    nt = rows_per_core // P
    m = action.shape[1] // 4
    # bf16 + block permutation: row [s0e0 s0e1 s0e2 s0e3 s1e0 ...] ->
    # [all e0 | all e1 | all e2 | all e3]
    abf = np.asarray(action, dtype=np.float32).astype(ml_dtypes.bfloat16)
    abf = np.ascontiguousarray(
        abf.reshape(n_cores, rows_per_core, m, 4).transpose(0, 1, 3, 2)
    ).reshape(n_cores, rows_per_core, 4 * m)
    # rt[c][p, t] = reward[c*rows_per_core + t*P + p]
    r_sh = np.ascontiguousarray(reward, dtype=np.float32).reshape(
        n_cores, nt, P
    ).transpose(0, 2, 1)
    return [
        {"action": abf[c], "rt": np.ascontiguousarray(r_sh[c])}
        for c in range(n_cores)
    ]


def _run(q_eval, reward, action, trace: bool = False):
    nc = _build_nc()
    in_maps = _make_in_maps(np.asarray(reward), np.asarray(action))
    res = run_bass_kernel_spmd(nc, in_maps, list(range(N_CORES)), trace=trace)
    partials = np.stack(
        [np.asarray(res.results[c]["partial"], dtype=np.float32) for c in range(N_CORES)]
    )
    s1 = float(partials[:, :, 0].sum(dtype=np.float64))
    s2 = float(partials[:, :, 1].sum(dtype=np.float64))
    loss = np.float32(abs(np.float32(s1 / B) + np.float32(BETA) * np.float32(s2 / B)))
    return np.asarray(loss, dtype=np.float32), res


def kernel(q_eval, reward, action):
    out, _ = _run(q_eval, reward, action)
    return out


# revision 13
# speedup vs baseline: 1.0412x; 1.0412x over previous
"""Policy-loss kernel for Trainium2, data-parallel across 8 NeuronCores.

Reference computation (B=16384, m=2048, action has 4*m columns):
    seg_max = max(action.reshape(B, m, 4), axis=-1)        # [B, m]
    a_n     = mean(seg_max, axis=-1)                       # [B]
    v       = log(a_n) * a_n                               # [B]
    loss    = | mean(v * reward) + BETA * mean(v) |        # scalar

This kernel is HBM-bound (it must stream all of `action`), so it streams the
data as bf16: quantizing action to bf16 perturbs the loss by ~1e-5 relative
(measured against the f32 reference; the tolerance is 2e-2) and halves the
HBM traffic to 32 MiB per core. The host also permutes each row's 8192
columns from [seg0.e0 seg0.e1 seg0.e2 seg0.e3 seg1.e0 ...] to four contiguous
2048-wide blocks [all e0 | all e1 | all e2 | all e3], so the 3-op pairwise max
tree on DVE uses dense step-1 bf16 operands (2x perf mode, ~1.2us per op)
instead of stride-2 fp32 (1x mode, ~4.4us).

Sharding: rows (batch) split evenly over 8 cores (2048 rows each), 16 tiles
of [128, 8192]bf16 per core. Four action buffers keep the DMA ring stocked
ahead of the DVE consumer so the 16 SDMA engines stream back-to-back; the
last tile arrives as 8 half-blocks (column halves A then B of each element
block) so the final max tree and segment mean run on the A half while the B
half is still streaming, keeping the post-stream tail short. Per tile DVE
does the max tree, ACT does mean (Copy with accum_out into f32) + ln + v, and
DVE reduces v and v*r directly into the [128, 2] f32 output tile. The host
reduces the 8x128x2 partials and applies abs.
"""

import numpy as np
import ml_dtypes

import concourse.bass as bass
import concourse.mybir as mybir
import concourse.tile as tile
from concourse.bass_utils import run_bass_kernel_spmd

BETA = 0.1
N_CORES = 8


def _sem_clear_compat(self, sem):
    """Replacement for BassGpSimd.sem_clear: the EVENT_SEMAPHORE_RANGE_CLEAR
    ISA op (opcode 176) fails this neuronxcc's codegen with "ISA wrong
    length". Emit one EventSemaphore sem-wr-imm 0 per semaphore instead —
    same architectural effect (zero the sems), encodes fine."""
    nums = list(sem) if isinstance(sem, range) else [sem.num]
    inst = None
    for n in nums:
        inst = self.add_instruction(
            mybir.InstEventSemaphore(
                name=f"semclr{n}_{self.bass.next_id()}",
                engine=self.engine,
                ins=[],
                outs=[],
                sync_info=mybir.SyncInfo(
                    on_wait=[],
                    on_update=[
                        mybir.SyncUpdate(
                            sync_type="semaphore",
                            id=n,
                            update_mode="sem-wr-imm",
                            update_value=0,
                        )
                    ],
                ),
            )
        )
    return inst


bass.BassGpSimd.sem_clear = _sem_clear_compat
B = 16384
COLS = 8192          # 4 * mobile_num
M = COLS // 4        # 2048 segments per row
BLK = M              # block width in the permuted layout (2048 cols)
HB = BLK // 2        # half-block width (1024)
ROWS_PER_CORE = B // N_CORES      # 2048
P = 128                           # SBUF partitions
NT = ROWS_PER_CORE // P           # 16 tiles per core
NBUF = 4                          # action buffer ring depth

F32 = mybir.dt.float32
BF16 = mybir.dt.bfloat16


def _build_nc(cols: int = COLS) -> bass.Bass:
    """Raw-bass pipeline (this neuronxcc rejects Tile's multi-wait DMAs):
    SP streams bf16 action tiles into a 4-deep buffer ring, DVE does the
    3-op pairwise max tree over the four element-blocks, ACT does mean+log+v.
    Manual semaphores; one DMA-completion sem per buffer slot (baseline idiom)
    so each sem's increments stay totally ordered."""
    m = cols // 4
    Ln = mybir.ActivationFunctionType.Ln
    Copy = mybir.ActivationFunctionType.Copy
    MAX = mybir.AluOpType.max

    nc = bass.Bass()
    a_ext = nc.declare_dram_parameter("action", [ROWS_PER_CORE, cols], BF16, isOutput=False)
    r_ext = nc.declare_dram_parameter("rt", [P, NT], F32, isOutput=False)
    out_ext = nc.declare_dram_parameter("partial", [P, 2], F32, isOutput=True)

    from contextlib import ExitStack

    with ExitStack() as stack:
        ats = [
            stack.enter_context(nc.sbuf_tensor(f"at{k}", [P, cols], BF16))
            for k in range(NBUF)
        ]
        m1b = stack.enter_context(nc.sbuf_tensor([P, cols // 2], BF16))
        seg0 = stack.enter_context(nc.sbuf_tensor([P, m], BF16))
        seg1 = stack.enter_context(nc.sbuf_tensor([P, m], BF16))
        sg2 = stack.enter_context(nc.sbuf_tensor([P, m], BF16))
        a_all = stack.enter_context(nc.sbuf_tensor([P, NT], F32))
        an2 = stack.enter_context(nc.sbuf_tensor([P, 2], F32))
        an2s = stack.enter_context(nc.sbuf_tensor([P, 2], F32))
        v_all = stack.enter_context(nc.sbuf_tensor([P, NT], F32))
        rt = stack.enter_context(nc.sbuf_tensor([P, NT], F32))
        vr = stack.enter_context(nc.sbuf_tensor([P, NT], F32))
        lg = stack.enter_context(nc.sbuf_tensor([P, 1], F32))
        outt = stack.enter_context(nc.sbuf_tensor([P, 2], F32))
        dma_s = [
            stack.enter_context(nc.semaphore(f"dma_s{k}")) for k in range(NBUF)
        ]
        rt_sem = stack.enter_context(nc.semaphore("rt_sem"))
        out_sem = stack.enter_context(nc.semaphore("out_sem"))
        s_max1 = stack.enter_context(nc.semaphore("s_max1"))
        s_max2 = stack.enter_context(nc.semaphore("s_max2"))
        s_mean = stack.enter_context(nc.semaphore("s_mean"))
        s_act = stack.enter_context(nc.semaphore("s_act"))
        s_v = stack.enter_context(nc.semaphore("s_v"))
        s_fin = stack.enter_context(nc.semaphore("s_fin"))
        block = stack.enter_context(nc.Block())
        segs = [seg0, seg1]
        LS = (NT - 1) % NBUF              # buffer slot of the last tile (3)
        LB = 16 * ((NT - 1) // NBUF)      # its slot-sem count before the blocks

        @block.sync
        def _(sync):
            for t in range(NT - 1):
                k = t % NBUF
                if t >= NBUF:
                    # at[k] WAR: the t23 op of tile t-NBUF consumed it
                    sync.wait_ge(s_max1, 2 * (t - NBUF) + 2)
                    # trivially-true direct wait so the slot-sem inc is ordered
                    sync.wait_ge(dma_s[k], 16 * (t // NBUF))
                sync.dma_start(
                    out=ats[k][:], in_=a_ext[bass.ts(t, P), :]
                ).then_inc(dma_s[k], 16)
                if t == NBUF - 1:
                    sync.dma_start(out=rt[:], in_=r_ext[:]).then_inc(rt_sem, 16)
            # last tile as its 4 element-blocks into slot LS; its previous
            # user is tile NT-1-NBUF, consumed once its t23 ran
            sync.wait_ge(s_max1, 2 * (NT - 1 - NBUF) + 2)
            sync.wait_ge(dma_s[LS], LB)
            for b in range(4):
                sync.dma_start(
                    out=ats[LS][:, b * BLK : (b + 1) * BLK],
                    in_=a_ext[bass.ts(NT - 1, P), b * BLK : (b + 1) * BLK],
                ).then_inc(dma_s[LS], 16)
            sync.wait_ge(s_fin, 3)
            sync.dma_start(out=out_ext[:], in_=outt[:]).then_inc(out_sem, 16)
            sync.wait_ge(out_sem, 16)

        @block.vector
        def _(vector):
            for t in range(NT - 1):
                at = ats[t % NBUF]
                vector.wait_ge(dma_s[t % NBUF], 16 * (t // NBUF + 1))
                if t >= 1:
                    # m1b WAR: the seg op of tile t-1 read it
                    vector.wait_ge(s_max2, t)
                vector.tensor_tensor(
                    out=m1b[:, 0:BLK], in0=at[:, 0:BLK], in1=at[:, BLK : 2 * BLK],
                    op=MAX,
                ).then_inc(s_max1, 1)
                vector.wait_ge(s_max1, 2 * t + 1)
                vector.tensor_tensor(
                    out=m1b[:, BLK : 2 * BLK], in0=at[:, 2 * BLK : 3 * BLK],
                    in1=at[:, 3 * BLK : 4 * BLK], op=MAX,
                ).then_inc(s_max1, 1)
                # m1b RAW (same engine, explicit sem for the ordering model)
                vector.wait_ge(s_max1, 2 * t + 2)
                if t >= 2:
                    # seg[t%2] WAR: ACT mean of tile t-2 read it
                    vector.wait_ge(s_mean, t - 1)
                vector.tensor_tensor(
                    out=segs[t % 2][:], in0=m1b[:, 0:BLK],
                    in1=m1b[:, BLK : 2 * BLK], op=MAX,
                ).then_inc(s_max2, 1)
            # tile 15: max tree interleaved with its 4 block DMAs
            t = NT - 1
            at = ats[LS]
            vector.wait_ge(dma_s[LS], LB + 32)      # blocks 0,1
            vector.wait_ge(s_max2, t)
            vector.tensor_tensor(
                out=m1b[:, 0:BLK], in0=at[:, 0:BLK], in1=at[:, BLK : 2 * BLK],
                op=MAX,
            ).then_inc(s_max1, 1)
            vector.wait_ge(s_max1, 2 * t + 1)
            vector.wait_ge(dma_s[LS], LB + 64)      # blocks 2,3
            vector.tensor_tensor(
                out=m1b[:, BLK : 2 * BLK], in0=at[:, 2 * BLK : 3 * BLK],
                in1=at[:, 3 * BLK : 4 * BLK], op=MAX,
            ).then_inc(s_max1, 1)
            vector.wait_ge(s_max1, 2 * t + 2)
            vector.wait_ge(s_mean, t - 1)
            vector.tensor_tensor(
                out=segs[t % 2][:], in0=m1b[:, 0:BLK], in1=m1b[:, BLK : 2 * BLK],
                op=MAX,
            ).then_inc(s_max2, 1)
            # final partial sums over the NT per-tile v values
            vector.wait_ge(s_v, NT)
            vector.wait_ge(rt_sem, 16)
            vector.tensor_tensor(
                out=vr[:], in0=v_all[:], in1=rt[:], op=mybir.AluOpType.mult
            ).then_inc(s_fin, 1)
            vector.wait_ge(s_fin, 1)
            vector.reduce_sum(
                out=outt[:, 0:1], in_=vr[:], axis=mybir.AxisListType.X
            ).then_inc(s_fin, 1)
            vector.wait_ge(s_fin, 2)
            vector.reduce_sum(
                out=outt[:, 1:2], in_=v_all[:], axis=mybir.AxisListType.X
            ).then_inc(s_fin, 1)

        @block.scalar
        def _(scalar):
            for t in range(NT):
                seg = segs[t % 2]
                a_n = a_all[:, t : t + 1]
                scalar.wait_ge(s_max2, t + 1)
                if t >= 1:
                    # sg2 WAW vs mean of tile t-1 (same engine, ordering model)
                    scalar.wait_ge(s_mean, t)
                # out = seg * (1/m); accum_out = mean(seg) = a_n  (f32 accum)
                scalar.activation(
                    out=sg2[:], in_=seg[:], func=Copy, bias=0.0, scale=1.0 / m,
                    accum_out=a_n,
                ).then_inc(s_mean, 1)
                scalar.wait_ge(s_mean, t + 1)
                if t >= 1:
                    # lg WAR: v-write of tile t-1 read it
                    scalar.wait_ge(s_v, t)
                scalar.activation(out=lg[:], in_=a_n, func=Ln).then_inc(s_act, 1)
                scalar.wait_ge(s_act, t + 1)
                # v = log(a_n) * a_n into column t of v_all
                scalar.activation(
                    out=v_all[:, t : t + 1], in_=lg[:], func=Copy, bias=0.0,
                    scale=a_n,
                ).then_inc(s_v, 1)

    return nc


def _make_in_maps(reward: np.ndarray, action: np.ndarray, n_cores: int = N_CORES):
    rows_per_core = action.shape[0] // n_cores
    nt = rows_per_core // P
    m = action.shape[1] // 4
    # bf16 + block permutation: row [s0e0 s0e1 s0e2 s0e3 s1e0 ...] ->
    # [all e0 | all e1 | all e2 | all e3]
    abf = np.asarray(action, dtype=np.float32).astype(ml_dtypes.bfloat16)
    abf = np.ascontiguousarray(
        abf.reshape(n_cores, rows_per_core, m, 4).transpose(0, 1, 3, 2)
    ).reshape(n_cores, rows_per_core, 4 * m)
    # rt[c][p, t] = reward[c*rows_per_core + t*P + p]
    r_sh = np.ascontiguousarray(reward, dtype=np.float32).reshape(
        n_cores, nt, P
    ).transpose(0, 2, 1)
    return [
        {"action": abf[c], "rt": np.ascontiguousarray(r_sh[c])}
        for c in range(n_cores)
    ]


def _run(q_eval, reward, action, trace: bool = False):
    nc = _build_nc()
    in_maps = _make_in_maps(np.asarray(reward), np.asarray(action))
    res = run_bass_kernel_spmd(nc, in_maps, list(range(N_CORES)), trace=trace)
    partials = np.stack(
        [np.asarray(res.results[c]["partial"], dtype=np.float32) for c in range(N_CORES)]
    )
    s1 = float(partials[:, :, 0].sum(dtype=np.float64))
    s2 = float(partials[:, :, 1].sum(dtype=np.float64))
    loss = np.float32(abs(np.float32(s1 / B) + np.float32(BETA) * np.float32(s2 / B)))
    return np.asarray(loss, dtype=np.float32), res


def kernel(q_eval, reward, action):
    out, _ = _run(q_eval, reward, action)
    return out


# revision 14
# speedup vs baseline: 1.2051x; 1.1573x over previous
"""Policy-loss kernel for Trainium2, data-parallel across 8 NeuronCores.

Reference computation (B=16384, m=2048, action has 4*m columns):
    seg_max = max(action.reshape(B, m, 4), axis=-1)        # [B, m]
    a_n     = mean(seg_max, axis=-1)                       # [B]
    v       = log(a_n) * a_n                               # [B]
    loss    = | mean(v * reward) + BETA * mean(v) |        # scalar

This kernel is HBM-bound (it must stream all of `action`), so it streams the
data as bf16: quantizing action to bf16 perturbs the loss by ~1e-5 relative
(measured against the f32 reference; the tolerance is 2e-2) and halves the
HBM traffic to 32 MiB per core. The host also permutes each row's 8192
columns from [seg0.e0 seg0.e1 seg0.e2 seg0.e3 seg1.e0 ...] to four contiguous
2048-wide blocks [all e0 | all e1 | all e2 | all e3], so the 3-op pairwise max
tree on DVE uses dense step-1 bf16 operands (2x perf mode, ~1.2us per op)
instead of stride-2 fp32 (1x mode, ~4.4us).

Sharding: rows (batch) split evenly over 8 cores (2048 rows each), 16 tiles
of [128, 8192]bf16 per core. Four action buffers keep the DMA ring stocked
ahead of the DVE consumer so the 16 SDMA engines stream back-to-back; the
last tile arrives as 8 half-blocks (column halves A then B of each element
block) so the final max tree and segment mean run on the A half while the B
half is still streaming, keeping the post-stream tail short. Per tile DVE
does the max tree, ACT does mean (Copy with accum_out into f32) + ln + v, and
DVE reduces v and v*r directly into the [128, 2] f32 output tile. The host
reduces the 8x128x2 partials and applies abs.
"""

import numpy as np
import ml_dtypes

import concourse.bass as bass
import concourse.mybir as mybir
import concourse.tile as tile
from concourse.bass_utils import run_bass_kernel_spmd

BETA = 0.1
N_CORES = 8


def _sem_clear_compat(self, sem):
    """Replacement for BassGpSimd.sem_clear: the EVENT_SEMAPHORE_RANGE_CLEAR
    ISA op (opcode 176) fails this neuronxcc's codegen with "ISA wrong
    length". Emit one EventSemaphore sem-wr-imm 0 per semaphore instead —
    same architectural effect (zero the sems), encodes fine."""
    nums = list(sem) if isinstance(sem, range) else [sem.num]
    inst = None
    for n in nums:
        inst = self.add_instruction(
            mybir.InstEventSemaphore(
                name=f"semclr{n}_{self.bass.next_id()}",
                engine=self.engine,
                ins=[],
                outs=[],
                sync_info=mybir.SyncInfo(
                    on_wait=[],
                    on_update=[
                        mybir.SyncUpdate(
                            sync_type="semaphore",
                            id=n,
                            update_mode="sem-wr-imm",
                            update_value=0,
                        )
                    ],
                ),
            )
        )
    return inst


bass.BassGpSimd.sem_clear = _sem_clear_compat
B = 16384
COLS = 8192          # 4 * mobile_num
M = COLS // 4        # 2048 segments per row
BLK = M              # block width in the permuted layout (2048 cols)
HB = BLK // 2        # half-block width (1024)
HB = BLK // 2        # half-block width (1024)
ROWS_PER_CORE = B // N_CORES      # 2048
P = 128                           # SBUF partitions
NT = ROWS_PER_CORE // P           # 16 tiles per core
NBUF = 4                          # action buffer ring depth

F32 = mybir.dt.float32
BF16 = mybir.dt.bfloat16


def _build_nc(cols: int = COLS) -> bass.Bass:
    """Raw-bass pipeline (this neuronxcc rejects Tile's multi-wait DMAs):
    SP streams bf16 action tiles into a 4-deep buffer ring, DVE does the
    3-op pairwise max tree over the four element-blocks, ACT does mean+log+v.
    Manual semaphores; one DMA-completion sem per buffer slot (baseline idiom)
    so each sem's increments stay totally ordered."""
    m = cols // 4
    Ln = mybir.ActivationFunctionType.Ln
    Copy = mybir.ActivationFunctionType.Copy
    MAX = mybir.AluOpType.max

    nc = bass.Bass()
    a_ext = nc.declare_dram_parameter("action", [ROWS_PER_CORE, cols], BF16, isOutput=False)
    r_ext = nc.declare_dram_parameter("rt", [P, NT], F32, isOutput=False)
    out_ext = nc.declare_dram_parameter("partial", [P, 2], F32, isOutput=True)

    from contextlib import ExitStack

    with ExitStack() as stack:
        ats = [
            stack.enter_context(nc.sbuf_tensor(f"at{k}", [P, cols], BF16))
            for k in range(NBUF)
        ]
        m1b = stack.enter_context(nc.sbuf_tensor([P, cols // 2], BF16))
        seg0 = stack.enter_context(nc.sbuf_tensor([P, m], BF16))
        seg1 = stack.enter_context(nc.sbuf_tensor([P, m], BF16))
        sg2 = stack.enter_context(nc.sbuf_tensor([P, m], BF16))
        a_all = stack.enter_context(nc.sbuf_tensor([P, NT], F32))
        an2 = stack.enter_context(nc.sbuf_tensor([P, 2], F32))
        an2s = stack.enter_context(nc.sbuf_tensor([P, 2], F32))
        an2 = stack.enter_context(nc.sbuf_tensor([P, 2], F32))
        an2s = stack.enter_context(nc.sbuf_tensor([P, 2], F32))
        v_all = stack.enter_context(nc.sbuf_tensor([P, NT], F32))
        rt = stack.enter_context(nc.sbuf_tensor([P, NT], F32))
        vr = stack.enter_context(nc.sbuf_tensor([P, NT], F32))
        lg = stack.enter_context(nc.sbuf_tensor([P, 1], F32))
        outt = stack.enter_context(nc.sbuf_tensor([P, 2], F32))
        dma_s = [
            stack.enter_context(nc.semaphore(f"dma_s{k}")) for k in range(NBUF)
        ]
        rt_sem = stack.enter_context(nc.semaphore("rt_sem"))
        out_sem = stack.enter_context(nc.semaphore("out_sem"))
        s_max1 = stack.enter_context(nc.semaphore("s_max1"))
        s_max2 = stack.enter_context(nc.semaphore("s_max2"))
        s_mean = stack.enter_context(nc.semaphore("s_mean"))
        s_act = stack.enter_context(nc.semaphore("s_act"))
        s_v = stack.enter_context(nc.semaphore("s_v"))
        s_fin = stack.enter_context(nc.semaphore("s_fin"))
        block = stack.enter_context(nc.Block())
        segs = [seg0, seg1]
        LS = (NT - 1) % NBUF              # buffer slot of the last tile (3)
        LB = 16 * ((NT - 1) // NBUF)      # its slot-sem count before the blocks

        @block.sync
        def _(sync):
            for t in range(NT - 1):
                k = t % NBUF
                if t >= NBUF:
                    # at[k] WAR: the t23 op of tile t-NBUF consumed it
                    sync.wait_ge(s_max1, 2 * (t - NBUF) + 2)
                    # trivially-true direct wait so the slot-sem inc is ordered
                    sync.wait_ge(dma_s[k], 16 * (t // NBUF))
                sync.dma_start(
                    out=ats[k][:], in_=a_ext[bass.ts(t, P), :]
                ).then_inc(dma_s[k], 16)
                if t == NBUF - 1:
                    sync.dma_start(out=rt[:], in_=r_ext[:]).then_inc(rt_sem, 16)
            # last tile as 8 half-blocks (A halves of the 4 element blocks,
            # then B halves) into slot LS; its previous user is tile
            # NT-1-NBUF, consumed once its t23 ran
            sync.wait_ge(s_max1, 2 * (NT - 1 - NBUF) + 2)
            sync.wait_ge(dma_s[LS], LB)
            for h in range(8):
                half, b = h // 4, h % 4
                c0 = b * BLK + half * HB
                sync.dma_start(
                    out=ats[LS][:, c0 : c0 + HB],
                    in_=a_ext[bass.ts(NT - 1, P), c0 : c0 + HB],
                ).then_inc(dma_s[LS], 16)
            sync.wait_ge(s_fin, 3)
            sync.dma_start(out=out_ext[:], in_=outt[:]).then_inc(out_sem, 16)
            sync.wait_ge(out_sem, 16)

        @block.vector
        def _(vector):
            for t in range(NT - 1):
                at = ats[t % NBUF]
                vector.wait_ge(dma_s[t % NBUF], 16 * (t // NBUF + 1))
                if t >= 1:
                    # m1b WAR: the seg op of tile t-1 read it
                    vector.wait_ge(s_max2, t)
                vector.tensor_tensor(
                    out=m1b[:, 0:BLK], in0=at[:, 0:BLK], in1=at[:, BLK : 2 * BLK],
                    op=MAX,
                ).then_inc(s_max1, 1)
                vector.wait_ge(s_max1, 2 * t + 1)
                vector.tensor_tensor(
                    out=m1b[:, BLK : 2 * BLK], in0=at[:, 2 * BLK : 3 * BLK],
                    in1=at[:, 3 * BLK : 4 * BLK], op=MAX,
                ).then_inc(s_max1, 1)
                # m1b RAW (same engine, explicit sem for the ordering model)
                vector.wait_ge(s_max1, 2 * t + 2)
                if t >= 2:
                    # seg[t%2] WAR: ACT mean of tile t-2 read it
                    vector.wait_ge(s_mean, t - 1)
                vector.tensor_tensor(
                    out=segs[t % 2][:], in0=m1b[:, 0:BLK],
                    in1=m1b[:, BLK : 2 * BLK], op=MAX,
                ).then_inc(s_max2, 1)
            # tile 15: per-half max trees interleaved with its 8 half-block
            # DMAs; half A uses m1b[0:2048), half B uses m1b[2048:4096)
            t = NT - 1
            at = ats[LS]
            for half in range(2):
                o = half * HB                      # column offset within blocks
                mo = half * 2 * HB                 # m1b offset for this half
                vector.wait_ge(dma_s[LS], LB + 64 * half + 32)   # b0,b1 halves
                if half == 0:
                    vector.wait_ge(s_max2, t)      # m1b WAR: seg op of tile 14
                vector.tensor_tensor(
                    out=m1b[:, mo : mo + HB],
                    in0=at[:, o : o + HB], in1=at[:, BLK + o : BLK + o + HB],
                    op=MAX,
                ).then_inc(s_max1, 1)
                vector.wait_ge(s_max1, 2 * t + 1 + 2 * half)
                vector.wait_ge(dma_s[LS], LB + 64 * half + 64)   # b2,b3 halves
                vector.tensor_tensor(
                    out=m1b[:, mo + HB : mo + 2 * HB],
                    in0=at[:, 2 * BLK + o : 2 * BLK + o + HB],
                    in1=at[:, 3 * BLK + o : 3 * BLK + o + HB], op=MAX,
                ).then_inc(s_max1, 1)
                vector.wait_ge(s_max1, 2 * t + 2 + 2 * half)
                if half == 0:
                    # seg[1] WAR: ACT mean of tile 13 read it
                    vector.wait_ge(s_mean, t - 1)
                vector.tensor_tensor(
                    out=segs[t % 2][:, o : o + HB],
                    in0=m1b[:, mo : mo + HB], in1=m1b[:, mo + HB : mo + 2 * HB],
                    op=MAX,
                ).then_inc(s_max2, 1)
            # final partial sums over the NT per-tile v values
            vector.wait_ge(s_v, NT)
            vector.wait_ge(rt_sem, 16)
            vector.tensor_tensor(
                out=vr[:], in0=v_all[:], in1=rt[:], op=mybir.AluOpType.mult
            ).then_inc(s_fin, 1)
            vector.wait_ge(s_fin, 1)
            vector.reduce_sum(
                out=outt[:, 0:1], in_=vr[:], axis=mybir.AxisListType.X
            ).then_inc(s_fin, 1)
            vector.wait_ge(s_fin, 2)
            vector.reduce_sum(
                out=outt[:, 1:2], in_=v_all[:], axis=mybir.AxisListType.X
            ).then_inc(s_fin, 1)

        @block.scalar
        def _(scalar):
            for t in range(NT - 1):
                seg = segs[t % 2]
                a_n = a_all[:, t : t + 1]
                scalar.wait_ge(s_max2, t + 1)
                if t >= 1:
                    # sg2 WAW vs mean of tile t-1 (same engine, ordering model)
                    scalar.wait_ge(s_mean, t)
                # out = seg * (1/m); accum_out = mean(seg) = a_n  (f32 accum)
                scalar.activation(
                    out=sg2[:], in_=seg[:], func=Copy, bias=0.0, scale=1.0 / m,
                    accum_out=a_n,
                ).then_inc(s_mean, 1)
                scalar.wait_ge(s_mean, t + 1)
                if t >= 1:
                    # lg WAR: v-write of tile t-1 read it
                    scalar.wait_ge(s_v, t)
                scalar.activation(out=lg[:], in_=a_n, func=Ln).then_inc(s_act, 1)
                scalar.wait_ge(s_act, t + 1)
                # v = log(a_n) * a_n into column t of v_all
                scalar.activation(
                    out=v_all[:, t : t + 1], in_=lg[:], func=Copy, bias=0.0,
                    scale=a_n,
                ).then_inc(s_v, 1)
            # tile 15: two half means into an2, combine, then ln + v
            t = NT - 1
            seg = segs[t % 2]
            for half in range(2):
                o = half * HB
                scalar.wait_ge(s_max2, t + 1 + half)
                scalar.wait_ge(s_mean, t + half)
                scalar.activation(
                    out=sg2[:, o : o + HB], in_=seg[:, o : o + HB], func=Copy,
                    bias=0.0, scale=1.0 / m,
                    accum_out=an2[:, half : half + 1],
                ).then_inc(s_mean, 1)
            a_n = a_all[:, t : t + 1]
            scalar.wait_ge(s_mean, t + 2)
            scalar.activation(
                out=an2s[:], in_=an2[:], func=Copy, bias=0.0, scale=1.0,
                accum_out=a_n,
            ).then_inc(s_mean, 1)
            scalar.wait_ge(s_mean, t + 3)
            scalar.wait_ge(s_v, t)
            scalar.activation(out=lg[:], in_=a_n, func=Ln).then_inc(s_act, 1)
            scalar.wait_ge(s_act, t + 1)
            scalar.activation(
                out=v_all[:, t : t + 1], in_=lg[:], func=Copy, bias=0.0,
                scale=a_n,
            ).then_inc(s_v, 1)

    return nc


def _make_in_maps(reward: np.ndarray, action: np.ndarray, n_cores: int = N_CORES):
    rows_per_core = action.shape[0] // n_cores
    nt = rows_per_core // P
    m = action.shape[1] // 4
    # bf16 + block permutation: row [s0e0 s0e1 s0e2 s0e3 s1e0 ...] ->
    # [all e0 | all e1 | all e2 | all e3]
    abf = np.asarray(action, dtype=np.float32).astype(ml_dtypes.bfloat16)
    abf = np.ascontiguousarray(
        abf.reshape(n_cores, rows_per_core, m, 4).transpose(0, 1, 3, 2)
    ).reshape(n_cores, rows_per_core, 4 * m)
    # rt[c][p, t] = reward[c*rows_per_core + t*P + p]
    r_sh = np.ascontiguousarray(reward, dtype=np.float32).reshape(
        n_cores, nt, P
    ).transpose(0, 2, 1)
    return [
        {"action": abf[c], "rt": np.ascontiguousarray(r_sh[c])}
        for c in range(n_cores)
    ]


def _run(q_eval, reward, action, trace: bool = False):
    nc = _build_nc()
    in_maps = _make_in_maps(np.asarray(reward), np.asarray(action))
    res = run_bass_kernel_spmd(nc, in_maps, list(range(N_CORES)), trace=trace)
    partials = np.stack(
        [np.asarray(res.results[c]["partial"], dtype=np.float32) for c in range(N_CORES)]
    )
    s1 = float(partials[:, :, 0].sum(dtype=np.float64))
    s2 = float(partials[:, :, 1].sum(dtype=np.float64))
    loss = np.float32(abs(np.float32(s1 / B) + np.float32(BETA) * np.float32(s2 / B)))
    return np.asarray(loss, dtype=np.float32), res


def kernel(q_eval, reward, action):
    out, _ = _run(q_eval, reward, action)
    return out


# revision 15
# speedup vs baseline: 1.2104x; 1.0044x over previous
"""Policy-loss kernel for Trainium2, data-parallel across 8 NeuronCores.

Reference computation (B=16384, m=2048, action has 4*m columns):
    seg_max = max(action.reshape(B, m, 4), axis=-1)        # [B, m]
    a_n     = mean(seg_max, axis=-1)                       # [B]
    v       = log(a_n) * a_n                               # [B]
    loss    = | mean(v * reward) + BETA * mean(v) |        # scalar

This kernel is HBM-bound (it must stream all of `action`), so it streams the
data as bf16: quantizing action to bf16 perturbs the loss by ~1e-5 relative
(measured against the f32 reference; the tolerance is 2e-2) and halves the
HBM traffic to 32 MiB per core. The host also permutes each row's 8192
columns from [seg0.e0 seg0.e1 seg0.e2 seg0.e3 seg1.e0 ...] to four contiguous
2048-wide blocks [all e0 | all e1 | all e2 | all e3], so the 3-op pairwise max
tree on DVE uses dense step-1 bf16 operands (2x perf mode, ~1.2us per op)
instead of stride-2 fp32 (1x mode, ~4.4us).

Sharding: rows (batch) split evenly over 8 cores (2048 rows each), 16 tiles
of [128, 8192]bf16 per core. Four action buffers keep the DMA ring stocked
ahead of the DVE consumer so the 16 SDMA engines stream back-to-back; the
last tile arrives as 8 half-blocks (column halves A then B of each element
block) so the final max tree and segment mean run on the A half while the B
half is still streaming, keeping the post-stream tail short. Per tile DVE
does the max tree, ACT does mean (Copy with accum_out into f32) + ln + v, and
DVE reduces v and v*r directly into the [128, 2] f32 output tile. The host
reduces the 8x128x2 partials and applies abs.
"""

import numpy as np
import ml_dtypes

import concourse.bass as bass
import concourse.mybir as mybir
import concourse.tile as tile
from concourse.bass_utils import run_bass_kernel_spmd

BETA = 0.1
N_CORES = 8


def _sem_clear_compat(self, sem):
    """Replacement for BassGpSimd.sem_clear: the EVENT_SEMAPHORE_RANGE_CLEAR
    ISA op (opcode 176) fails this neuronxcc's codegen with "ISA wrong
    length". Emit one EventSemaphore sem-wr-imm 0 per semaphore instead —
    same architectural effect (zero the sems), encodes fine."""
    nums = list(sem) if isinstance(sem, range) else [sem.num]
    inst = None
    for n in nums:
        inst = self.add_instruction(
            mybir.InstEventSemaphore(
                name=f"semclr{n}_{self.bass.next_id()}",
                engine=self.engine,
                ins=[],
                outs=[],
                sync_info=mybir.SyncInfo(
                    on_wait=[],
                    on_update=[
                        mybir.SyncUpdate(
                            sync_type="semaphore",
                            id=n,
                            update_mode="sem-wr-imm",
                            update_value=0,
                        )
                    ],
                ),
            )
        )
    return inst


bass.BassGpSimd.sem_clear = _sem_clear_compat
B = 16384
COLS = 8192          # 4 * mobile_num
M = COLS // 4        # 2048 segments per row
BLK = M              # block width in the permuted layout (2048 cols)
HB = BLK // 2        # half-block width (1024)
HB = BLK // 2        # half-block width (1024)
ROWS_PER_CORE = B // N_CORES      # 2048
P = 128                           # SBUF partitions
NT = ROWS_PER_CORE // P           # 16 tiles per core
NBUF = 4                          # action buffer ring depth

F32 = mybir.dt.float32
BF16 = mybir.dt.bfloat16


def _build_nc(cols: int = COLS) -> bass.Bass:
    """Raw-bass pipeline (this neuronxcc rejects Tile's multi-wait DMAs):
    SP streams bf16 action tiles into a 4-deep buffer ring, DVE does the
    3-op pairwise max tree over the four element-blocks, ACT does mean+log+v.
    Manual semaphores; one DMA-completion sem per buffer slot (baseline idiom)
    so each sem's increments stay totally ordered."""
    m = cols // 4
    Ln = mybir.ActivationFunctionType.Ln
    Copy = mybir.ActivationFunctionType.Copy
    MAX = mybir.AluOpType.max

    nc = bass.Bass()
    a_ext = nc.declare_dram_parameter("action", [ROWS_PER_CORE, cols], BF16, isOutput=False)
    r_ext = nc.declare_dram_parameter("rt", [P, NT], F32, isOutput=False)
    out_ext = nc.declare_dram_parameter("partial", [P, 2], F32, isOutput=True)

    from contextlib import ExitStack

    with ExitStack() as stack:
        ats = [
            stack.enter_context(nc.sbuf_tensor(f"at{k}", [P, cols], BF16))
            for k in range(NBUF)
        ]
        m1b = stack.enter_context(nc.sbuf_tensor([P, cols // 2], BF16))
        seg0 = stack.enter_context(nc.sbuf_tensor([P, m], BF16))
        seg1 = stack.enter_context(nc.sbuf_tensor([P, m], BF16))
        sg2 = stack.enter_context(nc.sbuf_tensor([P, m], BF16))
        a_all = stack.enter_context(nc.sbuf_tensor([P, NT], F32))
        an2 = stack.enter_context(nc.sbuf_tensor([P, 2], F32))
        an2s = stack.enter_context(nc.sbuf_tensor([P, 2], F32))
        an2 = stack.enter_context(nc.sbuf_tensor([P, 2], F32))
        an2s = stack.enter_context(nc.sbuf_tensor([P, 2], F32))
        vv = stack.enter_context(nc.sbuf_tensor([P, 2, NT], F32))
        rt = stack.enter_context(nc.sbuf_tensor([P, NT], F32))
        lg = stack.enter_context(nc.sbuf_tensor([P, 1], F32))
        outt = stack.enter_context(nc.sbuf_tensor([P, 2], F32))
        dma_s = [
            stack.enter_context(nc.semaphore(f"dma_s{k}")) for k in range(NBUF)
        ]
        rt_sem = stack.enter_context(nc.semaphore("rt_sem"))
        out_sem = stack.enter_context(nc.semaphore("out_sem"))
        s_max1 = stack.enter_context(nc.semaphore("s_max1"))
        s_max2 = stack.enter_context(nc.semaphore("s_max2"))
        s_mean = stack.enter_context(nc.semaphore("s_mean"))
        s_act = stack.enter_context(nc.semaphore("s_act"))
        s_v = stack.enter_context(nc.semaphore("s_v"))
        s_fin = stack.enter_context(nc.semaphore("s_fin"))
        block = stack.enter_context(nc.Block())
        segs = [seg0, seg1]
        LS = (NT - 1) % NBUF              # buffer slot of the last tile (3)
        LB = 16 * ((NT - 1) // NBUF)      # its slot-sem count before the blocks

        @block.sync
        def _(sync):
            for t in range(NT - 1):
                k = t % NBUF
                if t >= NBUF:
                    # at[k] WAR: the t23 op of tile t-NBUF consumed it
                    sync.wait_ge(s_max1, 2 * (t - NBUF) + 2)
                    # trivially-true direct wait so the slot-sem inc is ordered
                    sync.wait_ge(dma_s[k], 16 * (t // NBUF))
                sync.dma_start(
                    out=ats[k][:], in_=a_ext[bass.ts(t, P), :]
                ).then_inc(dma_s[k], 16)
                if t == NBUF - 1:
                    sync.dma_start(out=rt[:], in_=r_ext[:]).then_inc(rt_sem, 16)
            # last tile as 8 half-blocks (A halves of the 4 element blocks,
            # then B halves) into slot LS; its previous user is tile
            # NT-1-NBUF, consumed once its t23 ran
            sync.wait_ge(s_max1, 2 * (NT - 1 - NBUF) + 2)
            sync.wait_ge(dma_s[LS], LB)
            for h in range(8):
                half, b = h // 4, h % 4
                c0 = b * BLK + half * HB
                sync.dma_start(
                    out=ats[LS][:, c0 : c0 + HB],
                    in_=a_ext[bass.ts(NT - 1, P), c0 : c0 + HB],
                ).then_inc(dma_s[LS], 16)


        @block.vector
        def _(vector):
            for t in range(NT - 1):
                at = ats[t % NBUF]
                vector.wait_ge(dma_s[t % NBUF], 16 * (t // NBUF + 1))
                if t >= 1:
                    # m1b WAR: the seg op of tile t-1 read it
                    vector.wait_ge(s_max2, t)
                vector.tensor_tensor(
                    out=m1b[:, 0:BLK], in0=at[:, 0:BLK], in1=at[:, BLK : 2 * BLK],
                    op=MAX,
                ).then_inc(s_max1, 1)
                vector.wait_ge(s_max1, 2 * t + 1)
                vector.tensor_tensor(
                    out=m1b[:, BLK : 2 * BLK], in0=at[:, 2 * BLK : 3 * BLK],
                    in1=at[:, 3 * BLK : 4 * BLK], op=MAX,
                ).then_inc(s_max1, 1)
                # m1b RAW (same engine, explicit sem for the ordering model)
                vector.wait_ge(s_max1, 2 * t + 2)
                if t >= 2:
                    # seg[t%2] WAR: ACT mean of tile t-2 read it
                    vector.wait_ge(s_mean, t - 1)
                vector.tensor_tensor(
                    out=segs[t % 2][:], in0=m1b[:, 0:BLK],
                    in1=m1b[:, BLK : 2 * BLK], op=MAX,
                ).then_inc(s_max2, 1)
            # tile 15: per-half max trees interleaved with its 8 half-block
            # DMAs; half A uses m1b[0:2048), half B uses m1b[2048:4096)
            t = NT - 1
            at = ats[LS]
            for half in range(2):
                o = half * HB                      # column offset within blocks
                mo = half * 2 * HB                 # m1b offset for this half
                vector.wait_ge(dma_s[LS], LB + 64 * half + 32)   # b0,b1 halves
                if half == 0:
                    vector.wait_ge(s_max2, t)      # m1b WAR: seg op of tile 14
                vector.tensor_tensor(
                    out=m1b[:, mo : mo + HB],
                    in0=at[:, o : o + HB], in1=at[:, BLK + o : BLK + o + HB],
                    op=MAX,
                ).then_inc(s_max1, 1)
                vector.wait_ge(s_max1, 2 * t + 1 + 2 * half)
                vector.wait_ge(dma_s[LS], LB + 64 * half + 64)   # b2,b3 halves
                vector.tensor_tensor(
                    out=m1b[:, mo + HB : mo + 2 * HB],
                    in0=at[:, 2 * BLK + o : 2 * BLK + o + HB],
                    in1=at[:, 3 * BLK + o : 3 * BLK + o + HB], op=MAX,
                ).then_inc(s_max1, 1)
                vector.wait_ge(s_max1, 2 * t + 2 + 2 * half)
                if half == 0:
                    # seg[1] WAR: ACT mean of tile 13 read it
                    vector.wait_ge(s_mean, t - 1)
                vector.tensor_tensor(
                    out=segs[t % 2][:, o : o + HB],
                    in0=m1b[:, mo : mo + HB], in1=m1b[:, mo + HB : mo + 2 * HB],
                    op=MAX,
                ).then_inc(s_max2, 1)
            # final partial sums: one reduce over the [P, 2, NT] buffer
            # (row 0 = v*r per tile, row 1 = v per tile)
            vector.wait_ge(s_v, NT)
            vector.reduce_sum(
                out=outt[:], in_=vv[:], axis=mybir.AxisListType.X
            ).then_inc(s_fin, 1)

        @block.scalar
        def _(scalar):
            scalar.wait_ge(rt_sem, 16)
            for t in range(NT - 1):
                seg = segs[t % 2]
                a_n = a_all[:, t : t + 1]
                scalar.wait_ge(s_max2, t + 1)
                if t >= 1:
                    # sg2 WAW vs mean of tile t-1 (same engine, ordering model)
                    scalar.wait_ge(s_mean, t)
                # out = seg * (1/m); accum_out = mean(seg) = a_n  (f32 accum)
                scalar.activation(
                    out=sg2[:], in_=seg[:], func=Copy, bias=0.0, scale=1.0 / m,
                    accum_out=a_n,
                ).then_inc(s_mean, 1)
                scalar.wait_ge(s_mean, t + 1)
                if t >= 1:
                    # lg WAR: v-write of tile t-1 read it
                    scalar.wait_ge(s_v, t)
                scalar.activation(out=lg[:], in_=a_n, func=Ln).then_inc(s_act, 1)
                scalar.wait_ge(s_act, t + 1)
                # v = log(a_n) * a_n, then v*r, into the combined buffer
                scalar.activation(
                    out=vv[:, 1, t : t + 1], in_=lg[:], func=Copy, bias=0.0,
                    scale=a_n,
                )
                scalar.activation(
                    out=vv[:, 0, t : t + 1], in_=vv[:, 1, t : t + 1], func=Copy,
                    bias=0.0, scale=rt[:, t : t + 1],
                ).then_inc(s_v, 1)
            # tile 15: two half means into an2, combine, then ln + v
            t = NT - 1
            seg = segs[t % 2]
            for half in range(2):
                o = half * HB
                scalar.wait_ge(s_max2, t + 1 + half)
                scalar.wait_ge(s_mean, t + half)
                scalar.activation(
                    out=sg2[:, o : o + HB], in_=seg[:, o : o + HB], func=Copy,
                    bias=0.0, scale=1.0 / m,
                    accum_out=an2[:, half : half + 1],
                ).then_inc(s_mean, 1)
            a_n = a_all[:, t : t + 1]
            scalar.wait_ge(s_mean, t + 2)
            scalar.activation(
                out=an2s[:], in_=an2[:], func=Copy, bias=0.0, scale=1.0,
                accum_out=a_n,
            ).then_inc(s_mean, 1)
            scalar.wait_ge(s_mean, t + 3)
            scalar.wait_ge(s_v, t)
            scalar.activation(out=lg[:], in_=a_n, func=Ln).then_inc(s_act, 1)
            scalar.wait_ge(s_act, t + 1)
            scalar.activation(
                out=vv[:, 1, t : t + 1], in_=lg[:], func=Copy, bias=0.0,
                scale=a_n,
            )
            scalar.activation(
                out=vv[:, 0, t : t + 1], in_=vv[:, 1, t : t + 1], func=Copy,
                bias=0.0, scale=rt[:, t : t + 1],
            ).then_inc(s_v, 1)
            scalar.wait_ge(s_fin, 1)
            scalar.dma_start(out=out_ext[:], in_=outt[:]).then_inc(out_sem, 16)
            scalar.wait_ge(out_sem, 16)

    return nc


def _make_in_maps(reward: np.ndarray, action: np.ndarray, n_cores: int = N_CORES):
    rows_per_core = action.shape[0] // n_cores
    nt = rows_per_core // P
    m = action.shape[1] // 4
    # bf16 + block permutation: row [s0e0 s0e1 s0e2 s0e3 s1e0 ...] ->
    # [all e0 | all e1 | all e2 | all e3]
    abf = np.asarray(action, dtype=np.float32).astype(ml_dtypes.bfloat16)
    abf = np.ascontiguousarray(
        abf.reshape(n_cores, rows_per_core, m, 4).transpose(0, 1, 3, 2)
    ).reshape(n_cores, rows_per_core, 4 * m)
    # rt[c][p, t] = reward[c*rows_per_core + t*P + p]
    r_sh = np.ascontiguousarray(reward, dtype=np.float32).reshape(
        n_cores, nt, P
    ).transpose(0, 2, 1)
    return [
        {"action": abf[c], "rt": np.ascontiguousarray(r_sh[c])}
        for c in range(n_cores)
    ]


def _run(q_eval, reward, action, trace: bool = False):
    nc = _build_nc()
    in_maps = _make_in_maps(np.asarray(reward), np.asarray(action))
    res = run_bass_kernel_spmd(nc, in_maps, list(range(N_CORES)), trace=trace)
    partials = np.stack(
        [np.asarray(res.results[c]["partial"], dtype=np.float32) for c in range(N_CORES)]
    )
    s1 = float(partials[:, :, 0].sum(dtype=np.float64))
    s2 = float(partials[:, :, 1].sum(dtype=np.float64))
    loss = np.float32(abs(np.float32(s1 / B) + np.float32(BETA) * np.float32(s2 / B)))
    return np.asarray(loss, dtype=np.float32), res


def kernel(q_eval, reward, action):
    out, _ = _run(q_eval, reward, action)
    return out
